# revision 27
# baseline (speedup 1.0000x reference)
"""FlowNetC correlation kernel for Trainium2 (8 NeuronCores, batch-sharded).

out[b, d, y, x] = mean_c in1[b,c,y,x] * in2pad[b,c, y+2*dyi, x+2*dxi],
d = dyi*21 + dxi (441 displacements, stride-2, pad 20).

Parity decomposition: with stride-2 displacements, y/x parities decouple.
For q = y%2, p = x%2, Y = y//2, X = x//2 (grid 24x32) and the padded
in2 parity grid Y' in [0,44), X' in [0,52):
    out_g[E, D, Y, X] = sum_c in1g[c,Y,X] * in2g[c, Y+E, X+D]
a dense 2D correlation with a 21x21 displacement window.  Entries that
read in2 padding are identically zero -> only the interior 24x32 of
in2g participates; the host fills the zeros.

Device strategy (per core, 2 batch elements, 8 (b,q,p) groups):
  - host packs both inputs partition-major into one bf16 tensor
    [128 c_low, 16 (g,cc), 1536]: in1g flat [24,32] ++ in2g X-panels
    ([iX][YI, JX16], so every patch is one contiguous 128-wide slice).
  - interior patches (3x2 grid of 8x16, M=128 uniform) are the
    stationary matmul weights; the pairing window of in1g columns is
    streamed as rhs in <=512-column chunks, accumulating the two
    128-channel chunks into PSUM.
  - psum [128, N] blocks are scale-copied (1/256, f32->bf16) into
    per-2-group staging tiles [128, 6240] (DVE/ACT alternating) and
    DMAed out as 4 transfers with 12.5KB per-partition descriptors.
  - host gathers valid (E,D,Y,X) entries via a precomputed index map.
"""
import sys

sys.path.insert(0, "/opt/trn_rl_repo")

import numpy as np

N_CORES = 8
B_LOC = 2          # batch elements per core
C, H, W = 256, 48, 64
ND = 21            # displacements per axis
GY, GX = 24, 32    # parity grid
PAD = 10           # interior offset in padded parity coords

Y_BLOCKS = [(10, 8), (18, 8), (26, 8)]
X_BLOCKS = [(10, 16), (26, 16)]
X_POFF = [0, GY * 16]
IN1_LEN = GY * GX          # 768
SEG = 2 * IN1_LEN          # 1536 per (g,cc)
MAXN = 512                 # matmul N limit (one psum bank)

_cache = {}


def _plan():
    """Chunk list: each is one psum tile [128, n] with staging offset."""
    if "plan" in _cache:
        return _cache["plan"], _cache["tot"]
    plan = []
    off = 0
    for Y0, ay in Y_BLOCKS:
        yw0, yw1 = max(0, Y0 - 20), min(GY, Y0 + ay)
        for ix, (X0, bx) in enumerate(X_BLOCKS):
            xw0, xw1 = max(0, X0 - 20), min(GX, X0 + bx)
            nx = xw1 - xw0
            nyw = yw1 - yw0
            nchunk = -(-nyw * nx // MAXN)
            rows = -(-nyw // nchunk)
            y0c = 0
            while y0c < nyw:
                nyc = min(rows, nyw - y0c)
                plan.append(
                    dict(Y0=Y0, ay=ay, X0=X0, bx=bx, ix=ix, yw0=yw0, xw0=xw0,
                         nx=nx, y0c=y0c, nyc=nyc, m=ay * bx, n=nyc * nx,
                         off=off)
                )
                off += nyc * nx
                y0c += nyc
    _cache["plan"], _cache["tot"] = plan, off
    return plan, off


def _build_module():
    import concourse.bacc as bacc
    import concourse.bass as bass
    import concourse.mybir as mybir
    import concourse.tile as tile

    f32 = mybir.dt.float32
    bf16 = mybir.dt.bfloat16
    plan, tot = _plan()

    nc = bacc.Bacc(None, target_bir_lowering=False, debug=False)

    i12_d = nc.declare_dram_parameter("i12", [128, 16, SEG], bf16, isOutput=False)
    o_d = nc.declare_dram_parameter("o", [4, 128, 2 * tot], bf16, isOutput=True)

    with tile.TileContext(nc) as tc:
        with (
            tc.tile_pool(name="inp", bufs=1) as inp,
            tc.tile_pool(name="st", bufs=4) as stp,
            tc.tile_pool(name="ps", bufs=8, space=bass.MemorySpace.PSUM) as ps,
        ):
            it = inp.tile([128, 16, SEG], bf16, name="it", tag="it")
            # per-group input DMAs: each g's compute unblocks as soon as
            # its own 2 channel-chunks land. Issued from ACT, whose engine
            # init finishes ~3us earlier than Sync's, so the first
            # transfer starts sooner.
            for g in range(8):
                nc.scalar.dma_start(
                    it[:, 2 * g:2 * g + 2, :], i12_d[:, 2 * g:2 * g + 2, :]
                )

            for gp in range(4):
                st = stp.tile([128, 2 * tot], bf16, name=f"st{gp}", tag="st")
                for gh in range(2):
                    g = gp * 2 + gh
                    for ci, ch in enumerate(plan):
                        P = ps.tile([ch["m"], ch["n"]], f32, tag="P")
                        w0 = IN1_LEN + X_POFF[ch["ix"]] + (ch["Y0"] - PAD) * ch["bx"]
                        for cc in range(2):
                            gc = g * 2 + cc
                            lhsT = it[:, gc, w0:w0 + ch["m"]]
                            rhs = it[:, gc, 0:IN1_LEN].rearrange(
                                "c (y x) -> c y x", y=GY
                            )[
                                :, ch["yw0"] + ch["y0c"]:ch["yw0"] + ch["y0c"] + ch["nyc"],
                                ch["xw0"]:ch["xw0"] + ch["nx"],
                            ]
                            nc.tensor.matmul(
                                P[:], lhsT, rhs, start=(cc == 0), stop=(cc == 1)
                            )
                        dst = st[0:ch["m"],
                                 gh * tot + ch["off"]:gh * tot + ch["off"] + ch["n"]]
                        if ci % 2 == 0:
                            nc.vector.tensor_scalar_mul(dst, P[:], 1.0 / C)
                        else:
                            nc.scalar.mul(dst, P[:], 1.0 / C)
                    # drain each g's half as soon as its copies finish
                    # (sync is idle once the inputs are issued; HWDGE has
                    # lower per-DMA generation overhead than SWDGE)
                    nc.sync.dma_start(
                        o_d[gp, :, gh * tot:(gh + 1) * tot],
                        st[:, gh * tot:(gh + 1) * tot],
                    )
    nc.compile()
    return nc


def get_module():
    if "nc" not in _cache:
        _cache["nc"] = _build_module()
    return _cache["nc"]


def _build_idx():
    """IDX[E, D, Y, X] -> flat position in the per-(b,q,p) [128*tot] buffer,
    or -1 where the output is identically zero (in2 padding)."""
    if "idx" in _cache:
        return _cache["idx"]
    plan, tot = _plan()
    idx = np.full((ND, ND, GY, GX), -1, dtype=np.int64)
    Xs = np.arange(GX)
    for ch in plan:
        ys = ch["yw0"] + ch["y0c"]
        for JY in range(ch["ay"]):
            Yp = ch["Y0"] + JY
            for JX in range(ch["bx"]):
                Xp = ch["X0"] + JX
                part = JY * ch["bx"] + JX
                base = part * tot + ch["off"]
                for iy in range(ch["nyc"]):
                    Y = ys + iy
                    E = Yp - Y
                    if not (0 <= E < ND):
                        continue
                    X = Xs[ch["xw0"]:ch["xw0"] + ch["nx"]]
                    D = Xp - X
                    sel = (D >= 0) & (D < ND)
                    idx[E, D[sel], Y, X[sel]] = (
                        base + iy * ch["nx"] + (X[sel] - ch["xw0"])
                    )
    _cache["idx"] = idx
    return idx


def _split_parity(arr):
    """[B_LOC, C, H, W] -> [8(b,q,p), C, GY, GX]."""
    a = arr.reshape(B_LOC, C, GY, 2, GX, 2).transpose(0, 3, 5, 1, 2, 4)
    return a.reshape(8, C, GY, GX)


def kernel(input1: np.ndarray, input2: np.ndarray, _trace=False) -> np.ndarray:
    import ml_dtypes
    from concourse.bass_utils import run_bass_kernel_spmd

    bf16 = ml_dtypes.bfloat16
    nc = get_module()
    plan, tot = _plan()
    idx = _build_idx()
    valid = idx >= 0
    idx_c = np.where(valid, idx, 0)

    in_maps = []
    for c in range(N_CORES):
        sl = slice(c * B_LOC, (c + 1) * B_LOC)
        i1 = _split_parity(np.asarray(input1[sl], np.float32))  # [8,C,GY,GX]
        i2 = _split_parity(np.asarray(input2[sl], np.float32))
        # in2 X-panels: [8, C, iX, YI, JX16] -> flat 768
        i2p = i2.reshape(8, C, GY, 2, 16).transpose(0, 1, 3, 2, 4).reshape(8, C, IN1_LEN)
        both = np.concatenate(
            [i1.reshape(8, C, IN1_LEN), i2p], axis=-1
        )  # [8, C, 1536]
        # -> [128 c_low, 16 (g,cc), 1536]
        i12 = both.reshape(8, 2, 128, SEG).transpose(2, 0, 1, 3).reshape(128, 16, SEG)
        in_maps.append({"i12": np.ascontiguousarray(i12.astype(bf16))})
    res = run_bass_kernel_spmd(nc, in_maps, list(range(N_CORES)), trace=_trace)

    out = np.zeros((N_CORES * B_LOC, ND * ND, H, W), dtype=np.float32)
    for c in range(N_CORES):
        o = np.asarray(res.results[c]["o"]).astype(np.float32)  # [4,128,2*tot]
        for b in range(B_LOC):
            for q in range(2):
                for p in range(2):
                    g = (b * 2 + q) * 2 + p
                    buf = o[g // 2, :, (g % 2) * tot:(g % 2 + 1) * tot]
                    og = np.where(valid, buf.ravel()[idx_c], 0.0)
                    out[c * B_LOC + b, :, q::2, p::2] = og.reshape(
                        ND * ND, GY, GX
                    )
    if _trace:
        kernel.last_exec_time_ns = res.exec_time_ns
    return out


kernel.last_exec_time_ns = None


# revision 28
# speedup vs baseline: 1.1418x; 1.1418x over previous
"""FlowNetC correlation kernel for Trainium2 (8 NeuronCores, batch-sharded).

out[b, d, y, x] = mean_c in1[b,c,y,x] * in2pad[b,c, y+2*dyi, x+2*dxi],
d = dyi*21 + dxi (441 displacements, stride-2, pad 20).

Parity decomposition: with stride-2 displacements, y/x parities decouple.
For q = y%2, p = x%2, Y = y//2, X = x//2 (grid 24x32) and the padded
in2 parity grid Y' in [0,44), X' in [0,52):
    out_g[E, D, Y, X] = sum_c in1g[c,Y,X] * in2g[c, Y+E, X+D]
a dense 2D correlation with a 21x21 displacement window.  Entries that
read in2 padding are identically zero -> only the interior 24x32 of
in2g participates; the host fills the zeros.

Device strategy (per core, 2 batch elements, 8 (b,q,p) groups):
  - host packs both inputs partition-major into one bf16 tensor
    [128 c_low, 16 (g,cc), 1536]: in1g flat [24,32] ++ in2g X-panels
    ([iX][YI, JX16], so every patch is one contiguous 128-wide slice).
  - interior patches (3x2 grid of 8x16, M=128 uniform) are the
    stationary matmul weights; the pairing window of in1g columns is
    streamed as rhs in <=512-column chunks, accumulating the two
    128-channel chunks into PSUM.
  - psum [128, N] blocks are scale-copied (1/256, f32->bf16) into
    per-2-group staging tiles [128, 6240] (DVE/ACT alternating) and
    DMAed out as 4 transfers with 12.5KB per-partition descriptors.
  - host gathers valid (E,D,Y,X) entries via a precomputed index map.
"""
import sys

sys.path.insert(0, "/opt/trn_rl_repo")

import numpy as np

N_CORES = 8
B_LOC = 2          # batch elements per core
C, H, W = 256, 48, 64
ND = 21            # displacements per axis
GY, GX = 24, 32    # parity grid
PAD = 10           # interior offset in padded parity coords

Y_BLOCKS = [(10, 8), (18, 8), (26, 8)]
X_BLOCKS = [(10, 16), (26, 16)]
X_POFF = [0, GY * 16]
IN1_LEN = GY * GX          # 768
SEG = 2 * IN1_LEN          # 1536 per (g,cc)
MAXN = 512                 # matmul N limit (one psum bank)

_cache = {}


def _plan():
    """Chunk list: each is one psum tile [128, n] with staging offset."""
    if "plan" in _cache:
        return _cache["plan"], _cache["tot"]
    plan = []
    off = 0
    for Y0, ay in Y_BLOCKS:
        yw0, yw1 = max(0, Y0 - 20), min(GY, Y0 + ay)
        for ix, (X0, bx) in enumerate(X_BLOCKS):
            xw0, xw1 = max(0, X0 - 20), min(GX, X0 + bx)
            nx = xw1 - xw0
            nyw = yw1 - yw0
            nchunk = -(-nyw * nx // MAXN)
            rows = -(-nyw // nchunk)
            y0c = 0
            while y0c < nyw:
                nyc = min(rows, nyw - y0c)
                plan.append(
                    dict(Y0=Y0, ay=ay, X0=X0, bx=bx, ix=ix, yw0=yw0, xw0=xw0,
                         nx=nx, y0c=y0c, nyc=nyc, m=ay * bx, n=nyc * nx,
                         off=off)
                )
                off += nyc * nx
                y0c += nyc
    _cache["plan"], _cache["tot"] = plan, off
    return plan, off


def _build_module():
    import concourse.bacc as bacc
    import concourse.bass as bass
    import concourse.mybir as mybir
    import concourse.tile as tile

    f32 = mybir.dt.float32
    bf16 = mybir.dt.bfloat16
    plan, tot = _plan()

    nc = bacc.Bacc(None, target_bir_lowering=False, debug=False)

    i12_d = nc.declare_dram_parameter("i12", [128, 16, SEG], bf16, isOutput=False)
    o_d = nc.declare_dram_parameter("o", [4, 128, 2 * tot], bf16, isOutput=True)

    with tile.TileContext(nc) as tc:
        with (
            tc.tile_pool(name="inp", bufs=1) as inp,
            tc.tile_pool(name="st", bufs=4) as stp,
            tc.tile_pool(name="ps", bufs=8, space=bass.MemorySpace.PSUM) as ps,
        ):
            it = inp.tile([128, 16, SEG], bf16, name="it", tag="it")
            # per-group input DMAs: each g's compute unblocks as soon as
            # its own 2 channel-chunks land
            for g in range(8):
                nc.sync.dma_start(
                    it[:, 2 * g:2 * g + 2, :], i12_d[:, 2 * g:2 * g + 2, :]
                )

            for gp in range(4):
                st = stp.tile([128, 2 * tot], bf16, name=f"st{gp}", tag="st")
                for gh in range(2):
                    g = gp * 2 + gh
                    for ci, ch in enumerate(plan):
                        P = ps.tile([ch["m"], ch["n"]], f32, tag="P")
                        w0 = IN1_LEN + X_POFF[ch["ix"]] + (ch["Y0"] - PAD) * ch["bx"]
                        for cc in range(2):
                            gc = g * 2 + cc
                            lhsT = it[:, gc, w0:w0 + ch["m"]]
                            rhs = it[:, gc, 0:IN1_LEN].rearrange(
                                "c (y x) -> c y x", y=GY
                            )[
                                :, ch["yw0"] + ch["y0c"]:ch["yw0"] + ch["y0c"] + ch["nyc"],
                                ch["xw0"]:ch["xw0"] + ch["nx"],
                            ]
                            nc.tensor.matmul(
                                P[:], lhsT, rhs, start=(cc == 0), stop=(cc == 1)
                            )
                        dst = st[0:ch["m"],
                                 gh * tot + ch["off"]:gh * tot + ch["off"] + ch["n"]]
                        if ci % 2 == 0:
                            nc.vector.tensor_scalar_mul(dst, P[:], 1.0 / C)
                        else:
                            nc.scalar.mul(dst, P[:], 1.0 / C)
                    # drain each g's half as soon as its copies finish
                    # (sync is idle once the inputs are issued; HWDGE has
                    # lower per-DMA generation overhead than SWDGE)
                    nc.sync.dma_start(
                        o_d[gp, :, gh * tot:(gh + 1) * tot],
                        st[:, gh * tot:(gh + 1) * tot],
                    )
    nc.compile()
    return nc


def get_module():
    if "nc" not in _cache:
        _cache["nc"] = _build_module()
    return _cache["nc"]


def _build_idx():
    """IDX[E, D, Y, X] -> flat position in the per-(b,q,p) [128*tot] buffer,
    or -1 where the output is identically zero (in2 padding)."""
    if "idx" in _cache:
        return _cache["idx"]
    plan, tot = _plan()
    idx = np.full((ND, ND, GY, GX), -1, dtype=np.int64)
    Xs = np.arange(GX)
    for ch in plan:
        ys = ch["yw0"] + ch["y0c"]
        for JY in range(ch["ay"]):
            Yp = ch["Y0"] + JY
            for JX in range(ch["bx"]):
                Xp = ch["X0"] + JX
                part = JY * ch["bx"] + JX
                base = part * tot + ch["off"]
                for iy in range(ch["nyc"]):
                    Y = ys + iy
                    E = Yp - Y
                    if not (0 <= E < ND):
                        continue
                    X = Xs[ch["xw0"]:ch["xw0"] + ch["nx"]]
                    D = Xp - X
                    sel = (D >= 0) & (D < ND)
                    idx[E, D[sel], Y, X[sel]] = (
                        base + iy * ch["nx"] + (X[sel] - ch["xw0"])
                    )
    _cache["idx"] = idx
    return idx


def _split_parity(arr):
    """[B_LOC, C, H, W] -> [8(b,q,p), C, GY, GX]."""
    a = arr.reshape(B_LOC, C, GY, 2, GX, 2).transpose(0, 3, 5, 1, 2, 4)
    return a.reshape(8, C, GY, GX)


def kernel(input1: np.ndarray, input2: np.ndarray, _trace=False) -> np.ndarray:
    import ml_dtypes
    from concourse.bass_utils import run_bass_kernel_spmd

    bf16 = ml_dtypes.bfloat16
    nc = get_module()
    plan, tot = _plan()
    idx = _build_idx()
    valid = idx >= 0
    idx_c = np.where(valid, idx, 0)

    in_maps = []
    for c in range(N_CORES):
        sl = slice(c * B_LOC, (c + 1) * B_LOC)
        i1 = _split_parity(np.asarray(input1[sl], np.float32))  # [8,C,GY,GX]
        i2 = _split_parity(np.asarray(input2[sl], np.float32))
        # in2 X-panels: [8, C, iX, YI, JX16] -> flat 768
        i2p = i2.reshape(8, C, GY, 2, 16).transpose(0, 1, 3, 2, 4).reshape(8, C, IN1_LEN)
        both = np.concatenate(
            [i1.reshape(8, C, IN1_LEN), i2p], axis=-1
        )  # [8, C, 1536]
        # -> [128 c_low, 16 (g,cc), 1536]
        i12 = both.reshape(8, 2, 128, SEG).transpose(2, 0, 1, 3).reshape(128, 16, SEG)
        in_maps.append({"i12": np.ascontiguousarray(i12.astype(bf16))})
    res = run_bass_kernel_spmd(nc, in_maps, list(range(N_CORES)), trace=_trace)

    out = np.zeros((N_CORES * B_LOC, ND * ND, H, W), dtype=np.float32)
    for c in range(N_CORES):
        o = np.asarray(res.results[c]["o"]).astype(np.float32)  # [4,128,2*tot]
        for b in range(B_LOC):
            for q in range(2):
                for p in range(2):
                    g = (b * 2 + q) * 2 + p
                    buf = o[g // 2, :, (g % 2) * tot:(g % 2 + 1) * tot]
                    og = np.where(valid, buf.ravel()[idx_c], 0.0)
                    out[c * B_LOC + b, :, q::2, p::2] = og.reshape(
                        ND * ND, GY, GX
                    )
    if _trace:
        kernel.last_exec_time_ns = res.exec_time_ns
    return out


kernel.last_exec_time_ns = None


# revision 29
# speedup vs baseline: 1.1579x; 1.0140x over previous
"""FlowNetC correlation kernel for Trainium2 (8 NeuronCores, batch-sharded).

out[b, d, y, x] = mean_c in1[b,c,y,x] * in2pad[b,c, y+2*dyi, x+2*dxi],
d = dyi*21 + dxi (441 displacements, stride-2, pad 20).

Parity decomposition: with stride-2 displacements, y/x parities decouple.
For q = y%2, p = x%2, Y = y//2, X = x//2 (grid 24x32) and the padded
in2 parity grid Y' in [0,44), X' in [0,52):
    out_g[E, D, Y, X] = sum_c in1g[c,Y,X] * in2g[c, Y+E, X+D]
a dense 2D correlation with a 21x21 displacement window.  Entries that
read in2 padding are identically zero -> only the interior 24x32 of
in2g participates; the host fills the zeros.

Device strategy (per core, 2 batch elements, 8 (b,q,p) groups):
  - host packs both inputs partition-major into one bf16 tensor
    [128 c_low, 16 (g,cc), 1536]: in1g flat [24,32] ++ in2g X-panels
    ([iX][YI, JX16], so every patch is one contiguous 128-wide slice).
  - interior patches (3x2 grid of 8x16, M=128 uniform) are the
    stationary matmul weights; the pairing window of in1g columns is
    streamed as rhs in <=512-column chunks, accumulating the two
    128-channel chunks into PSUM.
  - psum [128, N] blocks are scale-copied (1/256, f32->bf16) into
    per-2-group staging tiles [128, 6240] (DVE/ACT alternating) and
    DMAed out as 4 transfers with 12.5KB per-partition descriptors.
  - host gathers valid (E,D,Y,X) entries via a precomputed index map.
"""
import sys

sys.path.insert(0, "/opt/trn_rl_repo")

import numpy as np

N_CORES = 8
B_LOC = 2          # batch elements per core
C, H, W = 256, 48, 64
ND = 21            # displacements per axis
GY, GX = 24, 32    # parity grid
PAD = 10           # interior offset in padded parity coords

Y_BLOCKS = [(10, 8), (18, 8), (26, 8)]
X_BLOCKS = [(10, 16), (26, 16)]
X_POFF = [0, GY * 16]
IN1_LEN = GY * GX          # 768
SEG = 2 * IN1_LEN          # 1536 per (g,cc)
MAXN = 512                 # matmul N limit (one psum bank)

_cache = {}


def _plan():
    """Chunk list: each is one psum tile [128, n] with staging offset."""
    if "plan" in _cache:
        return _cache["plan"], _cache["tot"]
    plan = []
    off = 0
    for Y0, ay in Y_BLOCKS:
        yw0, yw1 = max(0, Y0 - 20), min(GY, Y0 + ay)
        for ix, (X0, bx) in enumerate(X_BLOCKS):
            xw0, xw1 = max(0, X0 - 20), min(GX, X0 + bx)
            nx = xw1 - xw0
            nyw = yw1 - yw0
            nchunk = -(-nyw * nx // MAXN)
            rows = -(-nyw // nchunk)
            y0c = 0
            while y0c < nyw:
                nyc = min(rows, nyw - y0c)
                plan.append(
                    dict(Y0=Y0, ay=ay, X0=X0, bx=bx, ix=ix, yw0=yw0, xw0=xw0,
                         nx=nx, y0c=y0c, nyc=nyc, m=ay * bx, n=nyc * nx,
                         off=off)
                )
                off += nyc * nx
                y0c += nyc
    _cache["plan"], _cache["tot"] = plan, off
    return plan, off


def _build_module():
    import concourse.bacc as bacc
    import concourse.bass as bass
    import concourse.mybir as mybir
    import concourse.tile as tile

    f32 = mybir.dt.float32
    bf16 = mybir.dt.bfloat16
    plan, tot = _plan()

    nc = bacc.Bacc(None, target_bir_lowering=False, debug=False)

    i12_d = nc.declare_dram_parameter("i12", [128, 16, SEG], bf16, isOutput=False)
    o_d = nc.declare_dram_parameter("o", [4, 128, 2 * tot], bf16, isOutput=True)

    with tile.TileContext(nc) as tc:
        with (
            tc.tile_pool(name="inp", bufs=1) as inp,
            tc.tile_pool(name="st", bufs=4) as stp,
            tc.tile_pool(name="ps", bufs=8, space=bass.MemorySpace.PSUM) as ps,
        ):
            it = inp.tile([128, 16, SEG], bf16, name="it", tag="it")
            # per-group input DMAs: each g's compute unblocks as soon as
            # its own 2 channel-chunks land
            for g in range(8):
                nc.sync.dma_start(
                    it[:, 2 * g:2 * g + 2, :], i12_d[:, 2 * g:2 * g + 2, :]
                )

            for gp in range(4):
                st = stp.tile([128, 2 * tot], bf16, name=f"st{gp}", tag="st")
                for gh in range(2):
                    g = gp * 2 + gh
                    for ci, ch in enumerate(plan):
                        P = ps.tile([ch["m"], ch["n"]], f32, tag="P")
                        w0 = IN1_LEN + X_POFF[ch["ix"]] + (ch["Y0"] - PAD) * ch["bx"]
                        for cc in range(2):
                            gc = g * 2 + cc
                            lhsT = it[:, gc, w0:w0 + ch["m"]]
                            rhs = it[:, gc, 0:IN1_LEN].rearrange(
                                "c (y x) -> c y x", y=GY
                            )[
                                :, ch["yw0"] + ch["y0c"]:ch["yw0"] + ch["y0c"] + ch["nyc"],
                                ch["xw0"]:ch["xw0"] + ch["nx"],
                            ]
                            nc.tensor.matmul(
                                P[:], lhsT, rhs, start=(cc == 0), stop=(cc == 1)
                            )
                        dst = st[0:ch["m"],
                                 gh * tot + ch["off"]:gh * tot + ch["off"] + ch["n"]]
                        if ci % 2 == 0:
                            nc.vector.tensor_scalar_mul(dst, P[:], 1.0 / C)
                        else:
                            nc.scalar.mul(dst, P[:], 1.0 / C)
                # one DMA per 2-group staging tile: 12.5KB per-partition
                # descriptors amortize per-descriptor overhead; the drain
                # is queue-bound, not availability-bound, so the coarser
                # granularity costs nothing
                nc.sync.dma_start(o_d[gp], st[:])
    nc.compile()
    return nc


def get_module():
    if "nc" not in _cache:
        _cache["nc"] = _build_module()
    return _cache["nc"]


def _build_idx():
    """IDX[E, D, Y, X] -> flat position in the per-(b,q,p) [128*tot] buffer,
    or -1 where the output is identically zero (in2 padding)."""
    if "idx" in _cache:
        return _cache["idx"]
    plan, tot = _plan()
    idx = np.full((ND, ND, GY, GX), -1, dtype=np.int64)
    Xs = np.arange(GX)
    for ch in plan:
        ys = ch["yw0"] + ch["y0c"]
        for JY in range(ch["ay"]):
            Yp = ch["Y0"] + JY
            for JX in range(ch["bx"]):
                Xp = ch["X0"] + JX
                part = JY * ch["bx"] + JX
                base = part * tot + ch["off"]
                for iy in range(ch["nyc"]):
                    Y = ys + iy
                    E = Yp - Y
                    if not (0 <= E < ND):
                        continue
                    X = Xs[ch["xw0"]:ch["xw0"] + ch["nx"]]
                    D = Xp - X
                    sel = (D >= 0) & (D < ND)
                    idx[E, D[sel], Y, X[sel]] = (
                        base + iy * ch["nx"] + (X[sel] - ch["xw0"])
                    )
    _cache["idx"] = idx
    return idx


def _split_parity(arr):
    """[B_LOC, C, H, W] -> [8(b,q,p), C, GY, GX]."""
    a = arr.reshape(B_LOC, C, GY, 2, GX, 2).transpose(0, 3, 5, 1, 2, 4)
    return a.reshape(8, C, GY, GX)


def kernel(input1: np.ndarray, input2: np.ndarray, _trace=False) -> np.ndarray:
    import ml_dtypes
    from concourse.bass_utils import run_bass_kernel_spmd

    bf16 = ml_dtypes.bfloat16
    nc = get_module()
    plan, tot = _plan()
    idx = _build_idx()
    valid = idx >= 0
    idx_c = np.where(valid, idx, 0)

    in_maps = []
    for c in range(N_CORES):
        sl = slice(c * B_LOC, (c + 1) * B_LOC)
        i1 = _split_parity(np.asarray(input1[sl], np.float32))  # [8,C,GY,GX]
        i2 = _split_parity(np.asarray(input2[sl], np.float32))
        # in2 X-panels: [8, C, iX, YI, JX16] -> flat 768
        i2p = i2.reshape(8, C, GY, 2, 16).transpose(0, 1, 3, 2, 4).reshape(8, C, IN1_LEN)
        both = np.concatenate(
            [i1.reshape(8, C, IN1_LEN), i2p], axis=-1
        )  # [8, C, 1536]
        # -> [128 c_low, 16 (g,cc), 1536]
        i12 = both.reshape(8, 2, 128, SEG).transpose(2, 0, 1, 3).reshape(128, 16, SEG)
        in_maps.append({"i12": np.ascontiguousarray(i12.astype(bf16))})
    res = run_bass_kernel_spmd(nc, in_maps, list(range(N_CORES)), trace=_trace)

    out = np.zeros((N_CORES * B_LOC, ND * ND, H, W), dtype=np.float32)
    for c in range(N_CORES):
        o = np.asarray(res.results[c]["o"]).astype(np.float32)  # [4,128,2*tot]
        for b in range(B_LOC):
            for q in range(2):
                for p in range(2):
                    g = (b * 2 + q) * 2 + p
                    buf = o[g // 2, :, (g % 2) * tot:(g % 2 + 1) * tot]
                    og = np.where(valid, buf.ravel()[idx_c], 0.0)
                    out[c * B_LOC + b, :, q::2, p::2] = og.reshape(
                        ND * ND, GY, GX
                    )
    if _trace:
        kernel.last_exec_time_ns = res.exec_time_ns
    return out


kernel.last_exec_time_ns = None
